# revision 37
# baseline (speedup 1.0000x reference)
"""DispersionLoss (InfoNCE_l2 variant) on 8 Trainium2 NeuronCores.

Computes  log( E_{i!=j}[ exp(-||z_i - z_j||^2 / tau) ] )  for z [8192, 512] fp32.

Strategy (fp8 DoubleRow + bias-folded columns + fused DVE reduce)
-----------------------------------------------------------------
Let y = z * sqrt(2/tau).  exp(-||z_i-z_j||^2/tau) = exp(y_i.y_j + v_i + v_j)
with v_i = -||y_i||^2/2 (the relu clamp only matters on the diagonal, which
is corrected on the host).

Quantization: ydata = e4m3(S*y[:, :510]) with S=192.  The centered column
bias delta_j = v_j - vbar is folded INTO the matmul contraction via the two
freed dims (510, 511): lhs rows carry 240.0, rhs rows carry a greedy 2-term
e4m3 decomposition of S^2*delta_j/240.  So PSUM = S^2*(G~_ij + delta~_j) with
no extra instructions; ScalarE computes exp(PSUM/S^2) directly.

The 8192x8192 pair matrix is tiled into 16x16 blocks of 512x512; each
unordered block pair computed once (same rotation-SPMD coverage as before:
core c owns row blocks {c, c+8} x col blocks {c..c+8} / {c+8..c+15}).
Quads are [128-row strip x 4 col blocks] so one exp + one reduce covers 2048
columns sharing a single host row factor A_i = exp(vbar + delta~_i).

Engine split per quad:
  - TensorE: 8 fp8 DoubleRow matmuls (K=256 each) accumulate into a
    [128, 2048] psum tile (2 LDWEIGHTS; stationary reused across banks).
  - ScalarE: one Exp activation (scale=1/S^2) -> bf16 E in SBUF; the last
    two quads fold the row-sum into the activation accumulator so nothing
    trails the final ACT.
  - VectorE: one fused scalar_tensor_tensor (pairwise add + accum_out
    row-sum) per full quad; one 3D reduce for the leftover block.
  - Diag-containing quads compute an end-aligned trapezoid of the core's
    own block (mirror half skipped); host corrects the same-strip squares.
  - Host: row factors, weight-2 everywhere, subtracts the exactly-known
    (fp8 inputs are host-generated) double-counted cells, then log().

Measured: 84.8us (bf16 baseline) -> ~52us; ScalarE exp stream is the
critical path (~2us/quad), TensorE at fp8 peak (216ns/matmul), pipeline
fill ~13us (fixed preamble 6.4us + DMA + HAM warm-up), teardown ~4us.
"""

import math

import numpy as np
import ml_dtypes

TAU = 100.0
N = 8192
DIM = 512
NCORES = 8
BLK = 512
NBLK = 16
P = 128
S = 192.0            # fp8 data scale
NDATA = 510          # data dims (510, 511 carry the folded column bias)
NFULL = 16           # full quads per core
NSTAT = 20           # stats columns: 16 full + 4 leftover strips
N_WARMUP_MM = 7
# quads whose exp runs on the DVE via the Schraudolph bit-trick instead of
# ScalarE, shortening the saturated ACT stream.  Their PSUM-consuming
# affine is emitted ahead of the previous quad's (deferred) reduction so
# the strict-FIFO DVE queue frees the PSUM buffer promptly.
SCH_COLS = (6, 12)
A_SCH = 12102203.161561485   # 2^23 / ln2
B_SCH = 1064866805.0

_cache = {}


def _build_nc():
    import concourse.bacc as bacc
    import concourse.mybir as mybir
    from concourse.tile import TileContext

    f8 = mybir.dt.float8e4
    bf16 = mybir.dt.bfloat16
    f32 = mybir.dt.float32
    Exp = mybir.ActivationFunctionType.Exp
    add = mybir.AluOpType.add
    X = mybir.AxisListType.X
    DR = mybir.MatmulPerfMode.DoubleRow

    nc = bacc.Bacc(trn_type="TRN2")

    yl = nc.dram_tensor("yl", [2, P, 4, BLK], f8, kind="ExternalInput")
    yr = nc.dram_tensor("yr", [NBLK, P, 4, BLK], f8, kind="ExternalInput")
    stats = nc.dram_tensor("stats", [P, NSTAT], f32, kind="ExternalOutput")

    # schedule: (kind, lhs_local, strip, col_base, stats_col).  The small
    # quad goes first: it only needs yl[0]+yr[8] (512KB) so the pipeline
    # starts ~2us earlier than a full quad (1.25MB prerequisite).
    sched = (
        [("small",)]
        + [("full", 0, s, 0, s) for s in range(4)]
        + [("full", 0, s, 4, 4 + s) for s in range(4)]
        + [("full", 1, s, 8, 8 + s) for s in range(4)]
        + [("full", 1, s, 12, 12 + s) for s in range(4)]
    )

    with TileContext(nc) as tc:
        with (
            tc.tile_pool(name="persist", bufs=1) as pp,
            tc.tile_pool(name="equad", bufs=3) as ep,
            tc.tile_pool(name="psum", bufs=2, space="PSUM") as psp,
        ):
            ylt = [
                pp.tile([P, 4, BLK], f8, tag=f"yl_{r}", name=f"yl_{r}")
                for r in range(2)
            ]
            yrt = [
                pp.tile([P, 4, BLK], f8, tag=f"yr_{L}", name=f"yr_{L}")
                for L in range(NBLK)
            ]
            stats_t = pp.tile([P, NSTAT], f32, tag="stats", name="stats_t")
            wl = pp.tile([P, 2, P], f8, tag="wl", name="wl")
            wr = pp.tile([P, 2, BLK], f8, tag="wr", name="wr")
            wa_in = pp.tile([P, 8], bf16, tag="wa_in", name="wa_in")
            wa_out = pp.tile([P, 8], bf16, tag="wa_out", name="wa_out")

            # PE warm-up (HAM clock gate) + ACT exp-table preload on memset
            # data, issued while the first DMAs stream.  Memsets race on two
            # engines so the first LDWEIGHTS fires as early as possible;
            # enough warm matmuls to have K=8/8 by the time real data lands.
            nc.gpsimd.memset(wl[:], 0.0)
            nc.gpsimd.memset(wr[:], 0.0)
            nc.gpsimd.memset(wa_in[:], 0.0)
            wps = psp.tile([P, 4 * BLK], f32, tag="ps", name="warm_ps")
            for _ in range(N_WARMUP_MM):
                nc.tensor.matmul(
                    wps[:, :BLK], wl[:], wr[:], start=True, stop=True,
                    perf_mode=DR,
                )
            nc.scalar.activation(wa_out[:], wa_in[:], Exp)

            # DMAs in consumption order, full tiles (dispatch cost ~0.6us
            # each dominates half-splitting).  The small quad's two tiles
            # lead, one per HWDGE queue; the scalar queue only gets early
            # DMAs (it serves activations afterwards).
            nc.sync.dma_start(ylt[0][:], yl[0])
            nc.scalar.dma_start(yrt[8][:], yr[8])
            nc.sync.dma_start(yrt[0][:], yr[0])
            nc.scalar.dma_start(yrt[1][:], yr[1])
            nc.sync.dma_start(yrt[2][:], yr[2])
            nc.scalar.dma_start(yrt[3][:], yr[3])
            for L in range(4, 8):
                nc.sync.dma_start(yrt[L][:], yr[L])
            nc.sync.dma_start(ylt[1][:], yl[1])
            for L in range(9, NBLK):
                nc.sync.dma_start(yrt[L][:], yr[L])

            inv_s2 = 1.0 / (S * S)

            # Each full quad's DVE reduction is emitted one schedule slot
            # late so a Schraudolph quad's PSUM-consuming affine can enter
            # the (strict FIFO) DVE queue ahead of it.
            pending = [None]

            def flush_pending():
                if pending[0] is not None:
                    pending[0]()
                    pending[0] = None

            for item in sched:
                ps = psp.tile([P, 4 * BLK], f32, tag="ps", name="ps")
                if item[0] == "full":
                    _, rb, strip, base, col = item
                    # bank 0 of the diag-containing quads holds the core's
                    # own block: keep only cols >= 128*strip (end-aligned
                    # trapezoid); the redundant mirror half is never
                    # computed and the same-strip squares are corrected on
                    # the host.
                    is_diag = (rb == 0 and base == 0) or (rb == 1 and base == 8)
                    doff = strip * P if is_diag else 0
                    for kc in range(2):
                        lhs = ylt[rb][
                            :, 2 * kc : 2 * kc + 2, strip * P : (strip + 1) * P
                        ]
                        for b in range(4):
                            c0 = doff if b == 0 else 0
                            nc.tensor.matmul(
                                ps[:, b * BLK + c0 : (b + 1) * BLK],
                                lhs,
                                yrt[base + b][:, 2 * kc : 2 * kc + 2, c0:],
                                start=(kc == 0),
                                stop=(kc == 1),
                                perf_mode=DR,
                            )
                    if col in SCH_COLS:
                        # exp via fp32 affine + int32 convert + bitcast
                        # (Schraudolph), entirely on the vector engine.
                        # The affine goes on the DVE queue BEFORE the
                        # previous quad's deferred reduction.
                        ei = ep.tile(
                            [P, 4 * BLK], mybir.dt.int32, tag="ei",
                            name=f"ei_{col}",
                        )
                        nc.vector.tensor_scalar(
                            ei[:], ps[:], A_SCH / (S * S), B_SCH,
                            mybir.AluOpType.mult, mybir.AluOpType.add,
                        )
                        flush_pending()
                        ef = ei[:].bitcast(f32)
                        t1f = ep.tile(
                            [P, 2 * BLK], f32, tag="t1f", name=f"t1f_{col}"
                        )
                        nc.vector.scalar_tensor_tensor(
                            t1f[:],
                            ef[:, : 2 * BLK],
                            1.0,
                            ef[:, 2 * BLK :],
                            mybir.AluOpType.mult,
                            add,
                            accum_out=stats_t[:, col : col + 1],
                        )
                        continue
                    e = ep.tile([P, 4 * BLK], bf16, tag="e", name=f"e_{col}")
                    w = 4 * BLK - doff
                    if col >= 14:
                        # last quads: fold the row-sum into the activation
                        # (accumulator read ~0.3us) so nothing trails the
                        # final ACT but the stats DMA.
                        nc.scalar.activation(
                            e[:, :w], ps[:, doff:], Exp, scale=inv_s2,
                            accum_out=stats_t[:, col : col + 1],
                        )
                        flush_pending()
                    else:
                        nc.scalar.activation(
                            e[:, :w], ps[:, doff:], Exp, scale=inv_s2
                        )
                        flush_pending()

                        def _stt(e=e, w=w, col=col):
                            # one fused DVE op: t1 = e_lo + e_hi with
                            # accum_out = sum(t1) = the quad's row-sum
                            t1 = ep.tile(
                                [P, 2 * BLK], bf16, tag="t1", name=f"t1_{col}"
                            )
                            h = w // 2
                            nc.vector.scalar_tensor_tensor(
                                t1[:, :h],
                                e[:, :h],
                                1.0,
                                e[:, h:w],
                                mybir.AluOpType.mult,
                                add,
                                accum_out=stats_t[:, col : col + 1],
                            )

                        pending[0] = _stt
                else:
                    # leftover col block 8 vs the 4 strips of row block 0:
                    # bank b holds strip b, so the reduce is per-bank.
                    # bank-outer matmul order + split activation so the
                    # first ACT (head of the saturated ScalarE stream)
                    # fires after 4 matmuls instead of 8.
                    for b in range(4):
                        for kc in range(2):
                            nc.tensor.matmul(
                                ps[:, b * BLK : (b + 1) * BLK],
                                ylt[0][:, 2 * kc : 2 * kc + 2, b * P : (b + 1) * P],
                                yrt[8][:, 2 * kc : 2 * kc + 2, :],
                                start=(kc == 0),
                                stop=(kc == 1),
                                perf_mode=DR,
                            )
                    e = ep.tile([P, 4 * BLK], bf16, tag="e", name="e_small")
                    for h in range(2):
                        sl = slice(h * 2 * BLK, (h + 1) * 2 * BLK)
                        nc.scalar.activation(
                            e[:, sl], ps[:, sl], Exp, scale=inv_s2
                        )
                        nc.vector.reduce_sum(
                            stats_t[:, NFULL + 2 * h : NFULL + 2 * h + 2],
                            e[:, sl].rearrange("p (r b) -> p r b", r=2),
                            axis=X,
                        )

            flush_pending()
            # stats out in two pieces: cols 0..13 + small cols are done
            # before the last two (accum) quads finish, so that DMA hides;
            # only the tiny 14..15 slice trails the final activation, and it
            # rides the scalar engine's own HWDGE queue (no cross-engine
            # wakeup after the last accumulator read).
            nc.sync.dma_start(stats[:, : NFULL - 2], stats_t[:, : NFULL - 2])
            nc.scalar.dma_start(stats[:, NFULL - 2 :], stats_t[:, NFULL - 2 :])

    nc.compile()
    return nc


def _quantize_e4m3(x: np.ndarray) -> np.ndarray:
    """float64 -> TRN e4m3 (bias 7, max +-240) -> float64 of the stored value."""
    q = np.clip(x, -240.0, 240.0).astype(ml_dtypes.float8_e4m3)
    return q.astype(np.float64), q


def _host_prep(z: np.ndarray):
    """Quantize/fold inputs; returns per-core input maps + reduction data."""
    f8 = ml_dtypes.float8_e4m3
    z64 = z.astype(np.float64)
    y = z64 * math.sqrt(2.0 / TAU)            # [N, DIM]
    v = -0.5 * np.sum(y * y, axis=1)          # true v_i (all 512 dims)
    vbar = float(v.mean())
    delta = v - vbar

    # data dims quantized at scale S
    qdata64, qdata8 = _quantize_e4m3(S * y[:, :NDATA])   # [N, 510]

    # two-term e4m3 decomposition of S^2*delta/240 for the folded bias
    T = (S * S) * delta / 240.0
    b1_64, b1_8 = _quantize_e4m3(T)
    b2_64, b2_8 = _quantize_e4m3(T - b1_64)
    dtil = 240.0 * (b1_64 + b2_64) / (S * S)  # folded delta~ (exact)
    arow = np.exp(vbar + dtil)                # host row factors A_i

    # Q matrix [DIM, N] in fp8: data rows + 2 bias rows; lhs variant has 240s
    Qr = np.zeros((DIM, N), dtype=f8)
    Qr[:NDATA] = qdata8.T
    Qr[NDATA] = b1_8
    Qr[NDATA + 1] = b2_8
    Ql = Qr.copy()
    Ql[NDATA] = f8(240.0)
    Ql[NDATA + 1] = f8(240.0)

    def block_tile(Q, b):
        # [DIM, BLK] -> [kc=4, p=128, BLK] -> [128, 4, BLK]; dim = 128*kc + p
        blk = Q[:, b * BLK : (b + 1) * BLK]
        return np.ascontiguousarray(
            blk.reshape(4, P, BLK).transpose(1, 0, 2)
        )

    tiles_r = [block_tile(Qr, b) for b in range(NBLK)]

    in_maps = []
    for c in range(NCORES):
        yr_in = np.stack([tiles_r[(c + L) % NBLK] for L in range(NBLK)])
        yl_in = np.stack(
            [block_tile(Ql, c), block_tile(Ql, (c + 8) % NBLK)]
        )
        in_maps.append({"yl": yl_in, "yr": yr_in})

    # Schraudolph calibration: lambda[core][col] = E[S(x)/exp(x)] over the
    # quad's actual pair population (sampled; distribution-level, not bulk
    # recompute).  S mimics the DVE: fp32 affine -> int32 -> bitcast.
    a32 = np.float32(A_SCH / (S * S))
    b32 = np.float32(B_SCH)
    rng = np.random.default_rng(12345)
    nsamp = 4096
    qT = qdata64  # [N, NDATA]
    lam = np.ones((NCORES, NFULL))
    for c in range(NCORES):
        for col in SCH_COLS:
            if col < 8:
                rb_abs, base, strip = c, (0 if col < 4 else 4), col % 4
            else:
                rb_abs, base, strip = c + 8, (8 if col < 12 else 12), col % 4
            i = BLK * rb_abs + P * strip + rng.integers(0, P, nsamp)
            Ls = rng.integers(0, 4, nsamp)
            j = (
                BLK * ((c + base + Ls) % NBLK)
                + rng.integers(0, BLK, nsamp)
            )
            g = np.einsum("ij,ij->i", qT[i], qT[j]) / (S * S)
            arg = g + dtil[j]
            sv = (
                (arg.astype(np.float32) * (S * S) * a32 + b32)
                .astype(np.int32)
                .view(np.float32)
                .astype(np.float64)
            )
            lam[c, col] = sv.sum() / np.exp(arg).sum()

    # host-side diagonal-block correction, in u-units
    # u_ij = exp(G~_ij + dtil_i + dtil_j + 2 vbar), G~ from quantized data
    # dims.  With the end-aligned trapezoid device coverage of diag blocks,
    # the double-counted cells are exactly the 4 same-strip 128x128 squares
    # (plus the i=j diagonal once more): corr_B = sum(squares) + trace.
    corr = 0.0
    for b in range(NBLK):
        for s_ in range(4):
            cols = slice(b * BLK + s_ * P, b * BLK + (s_ + 1) * P)
            qb = qdata64[cols, :]                  # [P, 510]
            G = (qb @ qb.T) / (S * S)
            ee = np.exp(
                G + dtil[cols][None, :] + dtil[cols][:, None] + 2.0 * vbar
            )
            corr += ee.sum() + np.trace(ee)
    return in_maps, arow, vbar, corr, lam


def _reduce(results, arow, vbar, corr, lam) -> np.ndarray:
    """Draw = sum over cores/quads of 2 * dot(stats_col, A_rows)."""
    draw = 0.0
    for c, out_map in enumerate(results):
        st = out_map["stats"].astype(np.float64)  # [P, NSTAT]
        prng = np.arange(P)
        for q in range(NFULL):
            rb_abs = c if q < 8 else c + 8
            strip = q % 4
            rows = BLK * rb_abs + P * strip + prng
            draw += 2.0 * float(st[:, q] @ arow[rows]) / lam[c, q]
        for s_ in range(4):
            rows = BLK * c + P * s_ + prng
            draw += 2.0 * float(st[:, NFULL + s_] @ arow[rows])
    w = math.exp(vbar) * draw - corr
    mean = w / (float(N) * float(N - 1))
    return np.array(math.log(mean), dtype=np.float32)


def run(z: np.ndarray, trace: bool = False, tmpdir=None):
    from concourse.bass_utils import run_bass_kernel_spmd

    if "nc" not in _cache:
        _cache["nc"] = _build_nc()
    nc = _cache["nc"]
    in_maps, arow, vbar, corr, lam = _host_prep(np.asarray(z, dtype=np.float32))
    res = run_bass_kernel_spmd(
        nc, in_maps, core_ids=list(range(NCORES)), trace=trace, tmpdir=tmpdir
    )
    return _reduce(res.results, arow, vbar, corr, lam), res


def kernel(z: np.ndarray) -> np.ndarray:
    out, _ = run(z, trace=False)
    return out


# revision 38
# speedup vs baseline: 1.0318x; 1.0318x over previous
"""DispersionLoss (InfoNCE_l2 variant) on 8 Trainium2 NeuronCores.

Computes  log( E_{i!=j}[ exp(-||z_i - z_j||^2 / tau) ] )  for z [8192, 512] fp32.

Strategy (fp8 DoubleRow + bias-folded columns + fused DVE reduce)
-----------------------------------------------------------------
Let y = z * sqrt(2/tau).  exp(-||z_i-z_j||^2/tau) = exp(y_i.y_j + v_i + v_j)
with v_i = -||y_i||^2/2 (the relu clamp only matters on the diagonal, which
is corrected on the host).

Quantization: ydata = e4m3(S*y[:, :510]) with S=192.  The centered column
bias delta_j = v_j - vbar is folded INTO the matmul contraction via the two
freed dims (510, 511): lhs rows carry 240.0, rhs rows carry a greedy 2-term
e4m3 decomposition of S^2*delta_j/240.  So PSUM = S^2*(G~_ij + delta~_j) with
no extra instructions; ScalarE computes exp(PSUM/S^2) directly.

The 8192x8192 pair matrix is tiled into 16x16 blocks of 512x512; each
unordered block pair computed once (same rotation-SPMD coverage as before:
core c owns row blocks {c, c+8} x col blocks {c..c+8} / {c+8..c+15}).
Quads are [128-row strip x 4 col blocks] so one exp + one reduce covers 2048
columns sharing a single host row factor A_i = exp(vbar + delta~_i).

Engine split per quad:
  - TensorE: 8 fp8 DoubleRow matmuls (K=256 each) accumulate into a
    [128, 2048] psum tile (2 LDWEIGHTS; stationary reused across banks).
  - ScalarE: one Exp activation (scale=1/S^2) -> bf16 E in SBUF; the last
    two quads fold the row-sum into the activation accumulator so nothing
    trails the final ACT.
  - VectorE: one fused scalar_tensor_tensor (pairwise add + accum_out
    row-sum) per full quad; one 3D reduce for the leftover block.
  - Diag-containing quads compute an end-aligned trapezoid of the core's
    own block (mirror half skipped); host corrects the same-strip squares.
  - Host: row factors, weight-2 everywhere, subtracts the exactly-known
    (fp8 inputs are host-generated) double-counted cells, then log().

Measured: 84.8us (bf16 baseline) -> ~52us; ScalarE exp stream is the
critical path (~2us/quad), TensorE at fp8 peak (216ns/matmul), pipeline
fill ~13us (fixed preamble 6.4us + DMA + HAM warm-up), teardown ~4us.
"""

import math

import numpy as np
import ml_dtypes

TAU = 100.0
N = 8192
DIM = 512
NCORES = 8
BLK = 512
NBLK = 16
P = 128
S = 192.0            # fp8 data scale
NDATA = 510          # data dims (510, 511 carry the folded column bias)
NFULL = 16           # full quads per core
NSTAT = 20           # stats columns: 16 full + 4 leftover strips
N_WARMUP_MM = 7
# quads whose exp runs on the DVE via the Schraudolph bit-trick instead of
# ScalarE.  Measured on HW (twice, incl with the DVE-queue ordering fix):
# the fp32 affine reads PSUM at 1x (2.3us) vs the 1.97us activation it
# replaces, so with the 2-deep PSUM pool every such quad stalls the ACT
# stream by ~2us -- a net loss.  Keep empty.
SCH_COLS = ()
A_SCH = 12102203.161561485   # 2^23 / ln2
B_SCH = 1064866805.0

_cache = {}


def _build_nc():
    import concourse.bacc as bacc
    import concourse.mybir as mybir
    from concourse.tile import TileContext

    f8 = mybir.dt.float8e4
    bf16 = mybir.dt.bfloat16
    f32 = mybir.dt.float32
    Exp = mybir.ActivationFunctionType.Exp
    add = mybir.AluOpType.add
    X = mybir.AxisListType.X
    DR = mybir.MatmulPerfMode.DoubleRow

    nc = bacc.Bacc(trn_type="TRN2")

    yl = nc.dram_tensor("yl", [2, P, 4, BLK], f8, kind="ExternalInput")
    yr = nc.dram_tensor("yr", [NBLK, P, 4, BLK], f8, kind="ExternalInput")
    stats = nc.dram_tensor("stats", [P, NSTAT], f32, kind="ExternalOutput")

    # schedule: (kind, lhs_local, strip, col_base, stats_col).  The small
    # quad goes first: it only needs yl[0]+yr[8] (512KB) so the pipeline
    # starts ~2us earlier than a full quad (1.25MB prerequisite).
    sched = (
        [("small",)]
        + [("full", 0, s, 0, s) for s in range(4)]
        + [("full", 0, s, 4, 4 + s) for s in range(4)]
        + [("full", 1, s, 8, 8 + s) for s in range(4)]
        + [("full", 1, s, 12, 12 + s) for s in range(4)]
    )

    with TileContext(nc) as tc:
        with (
            tc.tile_pool(name="persist", bufs=1) as pp,
            tc.tile_pool(name="equad", bufs=3) as ep,
            tc.tile_pool(name="psum", bufs=2, space="PSUM") as psp,
        ):
            ylt = [
                pp.tile([P, 4, BLK], f8, tag=f"yl_{r}", name=f"yl_{r}")
                for r in range(2)
            ]
            yrt = [
                pp.tile([P, 4, BLK], f8, tag=f"yr_{L}", name=f"yr_{L}")
                for L in range(NBLK)
            ]
            stats_t = pp.tile([P, NSTAT], f32, tag="stats", name="stats_t")
            wl = pp.tile([P, 2, P], f8, tag="wl", name="wl")
            wr = pp.tile([P, 2, BLK], f8, tag="wr", name="wr")
            wa_in = pp.tile([P, 8], bf16, tag="wa_in", name="wa_in")
            wa_out = pp.tile([P, 8], bf16, tag="wa_out", name="wa_out")

            # PE warm-up (HAM clock gate) + ACT exp-table preload on memset
            # data, issued while the first DMAs stream.  Memsets race on two
            # engines so the first LDWEIGHTS fires as early as possible;
            # enough warm matmuls to have K=8/8 by the time real data lands.
            nc.gpsimd.memset(wl[:], 0.0)
            nc.gpsimd.memset(wr[:], 0.0)
            nc.gpsimd.memset(wa_in[:], 0.0)
            wps = psp.tile([P, 4 * BLK], f32, tag="ps", name="warm_ps")
            for _ in range(N_WARMUP_MM):
                nc.tensor.matmul(
                    wps[:, :BLK], wl[:], wr[:], start=True, stop=True,
                    perf_mode=DR,
                )
            nc.scalar.activation(wa_out[:], wa_in[:], Exp)

            # DMAs in consumption order, full tiles (dispatch cost ~0.6us
            # each dominates half-splitting).  The small quad's two tiles
            # lead, one per HWDGE queue; the scalar queue only gets early
            # DMAs (it serves activations afterwards).
            nc.sync.dma_start(ylt[0][:], yl[0])
            nc.scalar.dma_start(yrt[8][:], yr[8])
            nc.sync.dma_start(yrt[0][:], yr[0])
            nc.scalar.dma_start(yrt[1][:], yr[1])
            nc.sync.dma_start(yrt[2][:], yr[2])
            nc.scalar.dma_start(yrt[3][:], yr[3])
            for L in range(4, 8):
                nc.sync.dma_start(yrt[L][:], yr[L])
            nc.sync.dma_start(ylt[1][:], yl[1])
            for L in range(9, NBLK):
                nc.sync.dma_start(yrt[L][:], yr[L])

            inv_s2 = 1.0 / (S * S)

            # Each full quad's DVE reduction is emitted one schedule slot
            # late so a Schraudolph quad's PSUM-consuming affine can enter
            # the (strict FIFO) DVE queue ahead of it.
            pending = [None]

            def flush_pending():
                if pending[0] is not None:
                    pending[0]()
                    pending[0] = None

            for item in sched:
                ps = psp.tile([P, 4 * BLK], f32, tag="ps", name="ps")
                if item[0] == "full":
                    _, rb, strip, base, col = item
                    # bank 0 of the diag-containing quads holds the core's
                    # own block: keep only cols >= 128*strip (end-aligned
                    # trapezoid); the redundant mirror half is never
                    # computed and the same-strip squares are corrected on
                    # the host.
                    is_diag = (rb == 0 and base == 0) or (rb == 1 and base == 8)
                    doff = strip * P if is_diag else 0
                    for kc in range(2):
                        lhs = ylt[rb][
                            :, 2 * kc : 2 * kc + 2, strip * P : (strip + 1) * P
                        ]
                        for b in range(4):
                            c0 = doff if b == 0 else 0
                            nc.tensor.matmul(
                                ps[:, b * BLK + c0 : (b + 1) * BLK],
                                lhs,
                                yrt[base + b][:, 2 * kc : 2 * kc + 2, c0:],
                                start=(kc == 0),
                                stop=(kc == 1),
                                perf_mode=DR,
                            )
                    if col in SCH_COLS:
                        # exp via fp32 affine + int32 convert + bitcast
                        # (Schraudolph), entirely on the vector engine.
                        # The affine goes on the DVE queue BEFORE the
                        # previous quad's deferred reduction.
                        ei = ep.tile(
                            [P, 4 * BLK], mybir.dt.int32, tag="ei",
                            name=f"ei_{col}",
                        )
                        nc.vector.tensor_scalar(
                            ei[:], ps[:], A_SCH / (S * S), B_SCH,
                            mybir.AluOpType.mult, mybir.AluOpType.add,
                        )
                        flush_pending()
                        ef = ei[:].bitcast(f32)
                        t1f = ep.tile(
                            [P, 2 * BLK], f32, tag="t1f", name=f"t1f_{col}"
                        )
                        nc.vector.scalar_tensor_tensor(
                            t1f[:],
                            ef[:, : 2 * BLK],
                            1.0,
                            ef[:, 2 * BLK :],
                            mybir.AluOpType.mult,
                            add,
                            accum_out=stats_t[:, col : col + 1],
                        )
                        continue
                    e = ep.tile([P, 4 * BLK], bf16, tag="e", name=f"e_{col}")
                    w = 4 * BLK - doff
                    if col >= 14:
                        # last quads: fold the row-sum into the activation
                        # (accumulator read ~0.3us) so nothing trails the
                        # final ACT but the stats DMA.
                        nc.scalar.activation(
                            e[:, :w], ps[:, doff:], Exp, scale=inv_s2,
                            accum_out=stats_t[:, col : col + 1],
                        )
                        flush_pending()
                    else:
                        nc.scalar.activation(
                            e[:, :w], ps[:, doff:], Exp, scale=inv_s2
                        )
                        flush_pending()

                        def _stt(e=e, w=w, col=col):
                            # one fused DVE op: t1 = e_lo + e_hi with
                            # accum_out = sum(t1) = the quad's row-sum
                            t1 = ep.tile(
                                [P, 2 * BLK], bf16, tag="t1", name=f"t1_{col}"
                            )
                            h = w // 2
                            nc.vector.scalar_tensor_tensor(
                                t1[:, :h],
                                e[:, :h],
                                1.0,
                                e[:, h:w],
                                mybir.AluOpType.mult,
                                add,
                                accum_out=stats_t[:, col : col + 1],
                            )

                        pending[0] = _stt
                else:
                    # leftover col block 8 vs the 4 strips of row block 0:
                    # bank b holds strip b, so the reduce is per-bank.
                    # bank-outer matmul order + split activation so the
                    # first ACT (head of the saturated ScalarE stream)
                    # fires after 4 matmuls instead of 8.
                    for b in range(4):
                        for kc in range(2):
                            nc.tensor.matmul(
                                ps[:, b * BLK : (b + 1) * BLK],
                                ylt[0][:, 2 * kc : 2 * kc + 2, b * P : (b + 1) * P],
                                yrt[8][:, 2 * kc : 2 * kc + 2, :],
                                start=(kc == 0),
                                stop=(kc == 1),
                                perf_mode=DR,
                            )
                    e = ep.tile([P, 4 * BLK], bf16, tag="e", name="e_small")
                    for h in range(2):
                        sl = slice(h * 2 * BLK, (h + 1) * 2 * BLK)
                        nc.scalar.activation(
                            e[:, sl], ps[:, sl], Exp, scale=inv_s2
                        )
                        nc.vector.reduce_sum(
                            stats_t[:, NFULL + 2 * h : NFULL + 2 * h + 2],
                            e[:, sl].rearrange("p (r b) -> p r b", r=2),
                            axis=X,
                        )

            flush_pending()
            # stats out in two pieces: cols 0..13 + small cols are done
            # before the last two (accum) quads finish, so that DMA hides;
            # only the tiny 14..15 slice trails the final activation, and it
            # rides the scalar engine's own HWDGE queue (no cross-engine
            # wakeup after the last accumulator read).
            nc.sync.dma_start(stats[:, : NFULL - 2], stats_t[:, : NFULL - 2])
            nc.scalar.dma_start(stats[:, NFULL - 2 :], stats_t[:, NFULL - 2 :])

    nc.compile()
    return nc


def _quantize_e4m3(x: np.ndarray) -> np.ndarray:
    """float64 -> TRN e4m3 (bias 7, max +-240) -> float64 of the stored value."""
    q = np.clip(x, -240.0, 240.0).astype(ml_dtypes.float8_e4m3)
    return q.astype(np.float64), q


def _host_prep(z: np.ndarray):
    """Quantize/fold inputs; returns per-core input maps + reduction data."""
    f8 = ml_dtypes.float8_e4m3
    z64 = z.astype(np.float64)
    y = z64 * math.sqrt(2.0 / TAU)            # [N, DIM]
    v = -0.5 * np.sum(y * y, axis=1)          # true v_i (all 512 dims)
    vbar = float(v.mean())
    delta = v - vbar

    # data dims quantized at scale S
    qdata64, qdata8 = _quantize_e4m3(S * y[:, :NDATA])   # [N, 510]

    # two-term e4m3 decomposition of S^2*delta/240 for the folded bias
    T = (S * S) * delta / 240.0
    b1_64, b1_8 = _quantize_e4m3(T)
    b2_64, b2_8 = _quantize_e4m3(T - b1_64)
    dtil = 240.0 * (b1_64 + b2_64) / (S * S)  # folded delta~ (exact)
    arow = np.exp(vbar + dtil)                # host row factors A_i

    # Q matrix [DIM, N] in fp8: data rows + 2 bias rows; lhs variant has 240s
    Qr = np.zeros((DIM, N), dtype=f8)
    Qr[:NDATA] = qdata8.T
    Qr[NDATA] = b1_8
    Qr[NDATA + 1] = b2_8
    Ql = Qr.copy()
    Ql[NDATA] = f8(240.0)
    Ql[NDATA + 1] = f8(240.0)

    def block_tile(Q, b):
        # [DIM, BLK] -> [kc=4, p=128, BLK] -> [128, 4, BLK]; dim = 128*kc + p
        blk = Q[:, b * BLK : (b + 1) * BLK]
        return np.ascontiguousarray(
            blk.reshape(4, P, BLK).transpose(1, 0, 2)
        )

    tiles_r = [block_tile(Qr, b) for b in range(NBLK)]

    in_maps = []
    for c in range(NCORES):
        yr_in = np.stack([tiles_r[(c + L) % NBLK] for L in range(NBLK)])
        yl_in = np.stack(
            [block_tile(Ql, c), block_tile(Ql, (c + 8) % NBLK)]
        )
        in_maps.append({"yl": yl_in, "yr": yr_in})

    # Schraudolph calibration: lambda[core][col] = E[S(x)/exp(x)] over the
    # quad's actual pair population (sampled; distribution-level, not bulk
    # recompute).  S mimics the DVE: fp32 affine -> int32 -> bitcast.
    a32 = np.float32(A_SCH / (S * S))
    b32 = np.float32(B_SCH)
    rng = np.random.default_rng(12345)
    nsamp = 4096
    qT = qdata64  # [N, NDATA]
    lam = np.ones((NCORES, NFULL))
    for c in range(NCORES):
        for col in SCH_COLS:
            if col < 8:
                rb_abs, base, strip = c, (0 if col < 4 else 4), col % 4
            else:
                rb_abs, base, strip = c + 8, (8 if col < 12 else 12), col % 4
            i = BLK * rb_abs + P * strip + rng.integers(0, P, nsamp)
            Ls = rng.integers(0, 4, nsamp)
            j = (
                BLK * ((c + base + Ls) % NBLK)
                + rng.integers(0, BLK, nsamp)
            )
            g = np.einsum("ij,ij->i", qT[i], qT[j]) / (S * S)
            arg = g + dtil[j]
            sv = (
                (arg.astype(np.float32) * (S * S) * a32 + b32)
                .astype(np.int32)
                .view(np.float32)
                .astype(np.float64)
            )
            lam[c, col] = sv.sum() / np.exp(arg).sum()

    # host-side diagonal-block correction, in u-units
    # u_ij = exp(G~_ij + dtil_i + dtil_j + 2 vbar), G~ from quantized data
    # dims.  With the end-aligned trapezoid device coverage of diag blocks,
    # the double-counted cells are exactly the 4 same-strip 128x128 squares
    # (plus the i=j diagonal once more): corr_B = sum(squares) + trace.
    corr = 0.0
    for b in range(NBLK):
        for s_ in range(4):
            cols = slice(b * BLK + s_ * P, b * BLK + (s_ + 1) * P)
            qb = qdata64[cols, :]                  # [P, 510]
            G = (qb @ qb.T) / (S * S)
            ee = np.exp(
                G + dtil[cols][None, :] + dtil[cols][:, None] + 2.0 * vbar
            )
            corr += ee.sum() + np.trace(ee)
    return in_maps, arow, vbar, corr, lam


def _reduce(results, arow, vbar, corr, lam) -> np.ndarray:
    """Draw = sum over cores/quads of 2 * dot(stats_col, A_rows)."""
    draw = 0.0
    for c, out_map in enumerate(results):
        st = out_map["stats"].astype(np.float64)  # [P, NSTAT]
        prng = np.arange(P)
        for q in range(NFULL):
            rb_abs = c if q < 8 else c + 8
            strip = q % 4
            rows = BLK * rb_abs + P * strip + prng
            draw += 2.0 * float(st[:, q] @ arow[rows]) / lam[c, q]
        for s_ in range(4):
            rows = BLK * c + P * s_ + prng
            draw += 2.0 * float(st[:, NFULL + s_] @ arow[rows])
    w = math.exp(vbar) * draw - corr
    mean = w / (float(N) * float(N - 1))
    return np.array(math.log(mean), dtype=np.float32)


def run(z: np.ndarray, trace: bool = False, tmpdir=None):
    from concourse.bass_utils import run_bass_kernel_spmd

    if "nc" not in _cache:
        _cache["nc"] = _build_nc()
    nc = _cache["nc"]
    in_maps, arow, vbar, corr, lam = _host_prep(np.asarray(z, dtype=np.float32))
    res = run_bass_kernel_spmd(
        nc, in_maps, core_ids=list(range(NCORES)), trace=trace, tmpdir=tmpdir
    )
    return _reduce(res.results, arow, vbar, corr, lam), res


def kernel(z: np.ndarray) -> np.ndarray:
    out, _ = run(z, trace=False)
    return out
